# revision 13
# baseline (speedup 1.0000x reference)
"""Trainium2 Bass kernel for a decoder block (LN->attn->residual->LN->FFN->residual).

Sharding: 8 NeuronCores, data-parallel over tokens. Cores 0-3 handle batch 0,
cores 4-7 batch 1. Core 4*b + p owns the four 128-token chunks
{15-p, 8+p, 7-p, p} of its batch (reflected assignment), stored in x_loc in
DESCENDING chunk order. This balances causal attention work exactly: every
core needs j-chunk coverage {all 16, m<=11, m<=7, m<=3} for its four i-chunk
positions, so one uniform program processes, per j-chunk source slot
(rank r, pos rho), an i-position prefix of width 128*(4-rho). Total
40/64 of dense attention work, identical instruction stream on all cores.

q (feature-major) and v (row-major with a per-head ones column that makes the
softmax denominator fall out of the attention matmul) are exchanged within
each 4-core batch group either by:
  - USE_REMOTE=True: direct SBUF->SBUF remote_dma_broadcast pushes with
    XOR-relative destinations (slot d on the receiver holds data from rank
    own^d, so slot 0 is always local and the mapping is core-uniform), or
  - USE_REMOTE=False: collective_compute AllGathers via DRAM (slot d = rank d).
Causality (including the diagonal partial chunk and the padded chunk that the
uniform prefix over-covers) is applied as data via per-core mask tensors;
only the LAST i-position block of each slot ever needs masking (proven by the
chunk-group structure), so the mask multiply is one [128,2,128] op per slot.

The reference computes scores[i,j] = k_i . q_j, softmaxes over j with a j<=i
mask; so k plays the usual "query" role and q/v the context role. Logits are
bounded (|s|*scale < ~10), so softmax is exp(s)/sum without max subtraction.

Shapes (hardcoded): B=2, T=2048, C=1024, H=16 heads, D=64, F=4096.
"""

import sys

sys.path.insert(0, "/opt/trn_rl_repo")

import numpy as np
import ml_dtypes

import concourse.bass as bass
import concourse.bacc as bacc
import concourse.tile as tile
from concourse import mybir
from concourse.bass_utils import run_bass_kernel_spmd
from concourse.masks import make_identity

F32 = mybir.dt.float32
BF16 = mybir.dt.bfloat16
AF = mybir.ActivationFunctionType
OP = mybir.AluOpType

B, T, C = 2, 2048, 1024
H, D = 16, 64
F = 4 * C
EPS = 1e-3
N_CORES = 8
GROUPS = [[0, 1, 2, 3], [4, 5, 6, 7]]
RT = 512          # tokens per core
NT = RT // 128    # 4 local token chunks of 128
NC_F = C // 128   # 8 feature chunks
NF_F = F // 128   # 32 ffn feature chunks
SCALE = 1.0 / float(np.sqrt(D))
NSLOT = 16        # attention (source d, pos rho) slots

USE_REMOTE = True

# i-chunk ownership of group-rank p, in descending order (= i-position order)
def ownchunks(p):
    return [15 - p, 8 + p, 7 - p, p]

# slot order: source index d (0=own under XOR mapping), pos rho descending
# width so the first accumulation into PSUM is full width.
# width(rho) = 128 * (4 - rho_rank) where chunks at list index r0 have the
# HIGHEST chunk id: ownchunks[idx]: idx 0 -> n=1, idx 1 -> n=2, idx 2 -> n=3,
# idx 3 -> n=4. Process idx order [3, 2, 1, 0] per source.
SLOT_IDX_ORDER = [3, 2, 1, 0]
def slot_n(idx):
    return idx + 1  # number of active i-positions for j-chunks at that index

# AllGather payloads per rank (fallback path): qT [C, RT] + v_aug [RT, H*65]
QT_ELEMS = C * RT
VA_ELEMS = RT * H * 65


def build_nc(affine=True, use_remote=USE_REMOTE):
    nc = bacc.Bacc(None, target_bir_lowering=False)

    x_d = nc.dram_tensor("x_loc", [RT, C], BF16, kind="ExternalInput")
    wq_d = nc.dram_tensor("Wq", [C, C], BF16, kind="ExternalInput")
    wk_d = nc.dram_tensor("Wk", [C, C], BF16, kind="ExternalInput")
    wv_d = nc.dram_tensor("Wv", [C, C], BF16, kind="ExternalInput")
    w1_d = nc.dram_tensor("W1", [C, F], BF16, kind="ExternalInput")
    w2_d = nc.dram_tensor("W2", [F, C], BF16, kind="ExternalInput")
    bq_d = nc.dram_tensor("bq", [C], F32, kind="ExternalInput")
    bk_d = nc.dram_tensor("bk", [C], F32, kind="ExternalInput")
    bv_d = nc.dram_tensor("bv", [C], F32, kind="ExternalInput")
    b1_d = nc.dram_tensor("b1", [F], F32, kind="ExternalInput")
    b2_d = nc.dram_tensor("b2", [C], F32, kind="ExternalInput")
    g1_d = nc.dram_tensor("ln1_g", [C], F32, kind="ExternalInput")
    be1_d = nc.dram_tensor("ln1_b", [C], F32, kind="ExternalInput")
    g2_d = nc.dram_tensor("ln2_g", [C], F32, kind="ExternalInput")
    be2_d = nc.dram_tensor("ln2_b", [C], F32, kind="ExternalInput")
    # per-slot causal mask for the LAST active i-position block: [NSLOT,128,128]
    mask_d = nc.dram_tensor("masks", [NSLOT, 128, 128], BF16, kind="ExternalInput")
    y_d = nc.dram_tensor("y", [RT, C], F32, kind="ExternalOutput")

    if not use_remote:
        agq_in = nc.dram_tensor("agq_in", [QT_ELEMS], BF16)
        agq_out = nc.dram_tensor("agq_out", [4 * QT_ELEMS], BF16)
        agv_in = nc.dram_tensor("agv_in", [VA_ELEMS], BF16)
        agv_out = nc.dram_tensor("agv_out", [4 * VA_ELEMS], BF16)

    def bcast_row(dram_vec):
        # [n] dram vector -> [128, n] broadcast AP (partition step 0)
        return bass.AP(tensor=dram_vec.tensor, offset=dram_vec.offset,
                       ap=[[0, 128], dram_vec.ap[0]])

    with tile.TileContext(nc) as tc:
        with (
            tc.tile_pool(name="const", bufs=1) as const,
            tc.tile_pool(name="big", bufs=1) as big,
            tc.tile_pool(name="gath", bufs=1) as gath,
            tc.tile_pool(name="wpool", bufs=1) as wpool,
            tc.tile_pool(name="w1s", bufs=3) as w1s,
            tc.tile_pool(name="w2s", bufs=2) as w2s,
            tc.tile_pool(name="stats", bufs=4) as stats,
            tc.tile_pool(name="exs", bufs=3) as exs,
            tc.tile_pool(name="avs_pool", bufs=2) as avs_pool,
            tc.tile_pool(name="mm_ps", bufs=2, space="PSUM") as mm_ps,
            tc.tile_pool(name="st_ps", bufs=2, space="PSUM") as st_ps,
            tc.tile_pool(name="av_ps", bufs=2, space="PSUM") as av_ps,
        ):
            # ---- load x first (per chunk) so LN1 starts immediately ----
            x_sb = big.tile([128, NT, C], BF16, tag="x", name="x_first")
            for t in range(NT):
                nc.sync.dma_start(out=x_sb[:, t],
                                  in_=x_d.rearrange("(t p) c -> p t c", p=128)[:, t])

            # ---- constants ----
            ident_b = const.tile([128, 128], BF16, tag="ident_b")
            make_identity(nc, ident_b)
            ident_f = const.tile([128, 128], F32, tag="ident_f")
            make_identity(nc, ident_f)
            eps_t = const.tile([128, 1], F32, tag="eps")
            nc.vector.memset(eps_t, EPS)
            # warm the ACT function tables (Sqrt/Exp/Gelu) while x streams in
            warm = stats.tile([128, 1], F32, tag="rs")
            nc.scalar.activation(out=warm, in_=eps_t, func=AF.Sqrt)
            nc.scalar.activation(out=warm, in_=eps_t, func=AF.Exp)
            nc.scalar.activation(out=warm, in_=eps_t, func=AF.Gelu)

            g1_r = bv_r = None
            if affine:
                g1_r = const.tile([128, 2, C], F32, tag="lnr")
                nc.sync.dma_start(out=g1_r[:, 0], in_=bcast_row(g1_d[:]))
                nc.sync.dma_start(out=g1_r[:, 1], in_=bcast_row(be1_d[:]))
                bv_r = const.tile([128, C], F32, tag="rowr")
                nc.sync.dma_start(out=bv_r, in_=bcast_row(bv_d[:]))

            bq_t = const.tile([128, NC_F], F32, tag="bq")
            nc.sync.dma_start(out=bq_t, in_=bq_d.rearrange("(a p) -> p a", p=128))
            bk_t = const.tile([128, NC_F], F32, tag="bk")
            nc.sync.dma_start(out=bk_t, in_=bk_d.rearrange("(a p) -> p a", p=128))
            b1_t = const.tile([128, NF_F], F32, tag="b1")
            nc.sync.dma_start(out=b1_t, in_=b1_d.rearrange("(a p) -> p a", p=128))

            # causal masks (small now: 16 slots x 128 x 128)
            mask_sb = const.tile([128, NSLOT, 128], BF16, tag="masks")
            nc.sync.dma_start(out=mask_sb, in_=mask_d.rearrange("m p i -> p m i"))

            # remote-push semaphores (one per source distance d, per payload)
            if use_remote:
                sem_q = [nc.alloc_semaphore(f"qarr{d}") for d in range(1, 4)]
                sem_v = [nc.alloc_semaphore(f"varr{d}") for d in range(1, 4)]
                sem_loc = nc.alloc_semaphore("rdma_local")

            def layernorm(src_tile, h_out, g_pair):
                # src [128, C] f32 -> h_out [128, C] bf16; g_pair [128, 2, C]
                st6 = stats.tile([128, 2, 6], F32, tag="bnst")
                for s in range(2):
                    nc.vector.bn_stats(out=st6[:, s], in_=src_tile[:, 512 * s:512 * (s + 1)])
                mv = stats.tile([128, 2], F32, tag="bnmv")
                nc.vector.bn_aggr(out=mv, in_=st6)
                rs = stats.tile([128, 1], F32, tag="rs")
                nc.scalar.activation(out=rs, in_=mv[:, 1:2], func=AF.Sqrt, bias=eps_t)
                nc.vector.reciprocal(out=rs, in_=rs)
                nc.vector.tensor_scalar(out=h_out, in0=src_tile, scalar1=mv[:, 0:1],
                                        scalar2=rs, op0=OP.subtract, op1=OP.mult)
                if affine:
                    nc.vector.tensor_mul(out=h_out, in0=h_out, in1=g_pair[:, 0])
                    nc.vector.tensor_add(out=h_out, in0=h_out, in1=g_pair[:, 1])

            # ---- LN1 -> h1 bf16, transpose -> h1T ----
            h1 = big.tile([128, NT, C], BF16, tag="h_row")
            for t in range(NT):
                layernorm(x_sb[:, t], h1[:, t], g1_r)
            h1T = big.tile([128, NC_F, RT], BF16, tag="hT")
            for t in range(NT):
                for fc in range(NC_F):
                    pt = mm_ps.tile([128, 128], BF16, tag="mm")
                    nc.tensor.transpose(pt, h1[:, t, 128 * fc:128 * (fc + 1)], ident_b)
                    nc.vector.tensor_copy(out=h1T[:, fc, 128 * t:128 * (t + 1)], in_=pt)

            # ---- QKV (weights streamed); q then v first so their pushes /
            # gathers can run while k is still being computed ----
            qT = big.tile([128, NC_F, RT], BF16, tag="qT")
            kT = big.tile([128, NC_F, RT], BF16, tag="kT")

            def proj(w_d_, b_t, outT):
                w_view = w_d_.rearrange("(a p) c -> p a c", p=128)
                for co in range(NC_F):
                    w_t = w1s.tile([128, NC_F, 128], BF16, tag="w1t")
                    nc.sync.dma_start(out=w_t, in_=w_view[:, :, 128 * co:128 * (co + 1)])
                    ps = mm_ps.tile([128, RT], F32, tag="mm")
                    for ci in range(NC_F):
                        nc.tensor.matmul(ps, w_t[:, ci, :],
                                         h1T[:, ci, :], start=(ci == 0), stop=(ci == NC_F - 1))
                    nc.vector.tensor_scalar_add(out=outT[:, co, :], in0=ps,
                                                scalar1=b_t[:, co:co + 1])

            proj(wq_d, bq_t, qT)

            # v row-major with interleaved per-head ones col: [tok%128, t, H, 65]
            v_aug = big.tile([128, NT, H, 65], BF16, tag="v_aug")
            nc.vector.memset(v_aug[:, :, :, 64:65], 1.0)
            wv_view = wv_d.rearrange("(a p) c -> p a c", p=128)
            for qt in range(4):
                wv_t = wpool.tile([128, NC_F, 256], BF16, tag="wvh", bufs=2)
                nc.sync.dma_start(out=wv_t, in_=wv_view[:, :, 256 * qt:256 * (qt + 1)])
                for t in range(NT):
                    ps = mm_ps.tile([128, 256], F32, tag="mm")
                    for ci in range(NC_F):
                        nc.tensor.matmul(ps, h1T[:, ci, 128 * t:128 * (t + 1)],
                                         wv_t[:, ci, :],
                                         start=(ci == 0), stop=(ci == NC_F - 1))
                    if affine:
                        nc.vector.tensor_tensor(
                            out=v_aug[:, t, 4 * qt:4 * (qt + 1), 0:64],
                            in0=ps.rearrange("p (a b) -> p a b", b=64),
                            in1=bv_r[:, 256 * qt:256 * (qt + 1)].rearrange(
                                "p (a b) -> p a b", b=64),
                            op=OP.add)
                    else:
                        nc.vector.tensor_copy(
                            out=v_aug[:, t, 4 * qt:4 * (qt + 1), 0:64],
                            in_=ps.rearrange("p (a b) -> p a b", b=64))

            # ---- exchange q and v within the 4-core batch group ----
            if use_remote:
                # gathered slots d=1..3 hold data from rank own^d
                q_g = gath.tile([128, 3, NC_F, RT], BF16, tag="q_g")
                v_g = gath.tile([128, 3, NT, H * 65], BF16, tag="v_g")

                def push(in_full, out_slot_fn, sems):
                    # two half-pushes per distance d on lane slots d and d+4
                    for di, d in enumerate((1, 2, 3)):
                        for hf in range(2):
                            rdests = [None] * 8
                            rdests[d + 4 * hf] = (0, d)
                            nc.gpsimd.remote_dma_broadcast(
                                out_ap=out_slot_fn(di, hf),
                                in_ap=in_full(hf),
                                remote_sem=sems[di],
                                local_sem=sem_loc,
                                rdests=rdests)
                    nc.gpsimd.trigger_dma(count=6)

                push(lambda hf: qT[:, 4 * hf:4 * (hf + 1), :],
                     lambda di, hf: q_g[:, di, 4 * hf:4 * (hf + 1), :], sem_q)
                push(lambda hf: v_aug[:, 2 * hf:2 * (hf + 1)],
                     lambda di, hf: v_g[:, di, 2 * hf:2 * (hf + 1), :], sem_v)
            else:
                agq_view = agq_in[:].rearrange("(f p t) -> p f t", p=128, t=RT)
                for co in range(NC_F):
                    nc.sync.dma_start(out=agq_view[:, co], in_=qT[:, co])
                nc.gpsimd.collective_compute(
                    "AllGather", OP.bypass, replica_groups=GROUPS,
                    ins=[agq_in[:]], outs=[agq_out[:]])
                agv_view = agv_in[:].rearrange("(tc p x) -> p tc x", p=128, x=H * 65)
                for t in range(NT):
                    nc.sync.dma_start(out=agv_view[:, t],
                                      in_=v_aug[:, t].rearrange("p b c -> p (b c)"))
                nc.gpsimd.collective_compute(
                    "AllGather", OP.bypass, replica_groups=GROUPS,
                    ins=[agv_in[:]], outs=[agv_out[:]])
                # land gathered data in SBUF (4 rank slots)
                q_g = gath.tile([128, 4, NC_F, RT], BF16, tag="q_g")
                v_g = gath.tile([128, 4, NT, H * 65], BF16, tag="v_g")
                nc.sync.dma_start(
                    out=q_g, in_=agq_out.rearrange("(r f p t) -> p r f t", p=128, t=RT, r=4))
                nc.sync.dma_start(
                    out=v_g, in_=agv_out.rearrange("(r tc p x) -> p r tc x", p=128, x=H * 65, r=4))

            # ---- k projection (no exchange needed; overlaps q/v comms) ----
            proj(wk_d, bk_t, kT)

            # prefetch first FFN1 weight tiles (consumed after attention)
            w1_view = w1_d.rearrange("(a p) f -> p a f", p=128)
            w1_pre = []
            for fo in range(2):
                w1_t = w1s.tile([128, NC_F, 128], BF16, tag="w1t",
                                name=f"w1pre_{fo}")
                nc.sync.dma_start(out=w1_t,
                                  in_=w1_view[:, :, 128 * fo:128 * (fo + 1)])
                w1_pre.append(w1_t)

            # ---- attention: per head-pair a, 16 (source, pos) slots with
            # i-position prefix widths; causality via mask data ----
            attn_sb = big.tile([128, NT, C], BF16, tag="attn_out")

            # slot descriptors: (src d, chunk idx, width, first, last)
            slots = []
            nsrc = 4
            for dsi in range(nsrc):
                for idx in SLOT_IDX_ORDER:
                    slots.append((dsi, idx))

            def q_stat(dsi, idx, a, hi):
                if use_remote and dsi == 0:
                    base = qT[64 * hi:64 * hi + 64, a, :]
                else:
                    si = dsi - 1 if use_remote else dsi
                    base = q_g[64 * hi:64 * hi + 64, si, a, :]
                return base[:, 128 * idx:128 * (idx + 1)]

            def v_stat(dsi, idx, a, hi):
                h = 2 * a + hi
                if use_remote and dsi == 0:
                    return v_aug[:, idx, h, :]
                si = dsi - 1 if use_remote else dsi
                return v_g[:, si, idx, 65 * h:65 * (h + 1)]

            rdma_waiters = []

            for a in range(H // 2):
                avs2 = [av_ps.tile([65, RT], F32, tag="av", name=f"av_{a}_{k2}")
                        for k2 in range(2)]
                pend = None  # software pipeline: AV lags score by one slot
                for s_i, (dsi, idx) in enumerate(slots):
                    n = slot_n(idx)
                    w = 128 * n
                    st = st_ps.tile([128, 2, RT], F32, tag="st")
                    for hi in range(2):
                        mm = nc.tensor.matmul(st[:, hi, 0:w], q_stat(dsi, idx, a, hi),
                                              kT[64 * hi:64 * hi + 64, a, 0:w],
                                              start=True, stop=True,
                                              tile_position=(64 * hi, 0))
                        if use_remote and dsi > 0:
                            rdma_waiters.append((mm, sem_q[dsi - 1]))
                    ex = exs.tile([128, 2, RT], BF16, tag="ex")
                    nc.scalar.activation(out=ex[:, :, 0:w], in_=st[:, :, 0:w],
                                         func=AF.Exp, scale=SCALE)
                    # mask the last active i-position block (gpsimd)
                    msl = mask_sb[:, s_i, :]
                    for hi in range(2):
                        nc.gpsimd.tensor_mul(out=ex[:, hi, w - 128:w],
                                             in0=ex[:, hi, w - 128:w], in1=msl)
                    if pend is not None:
                        p_dsi, p_idx, p_w, p_ex, p_first = pend
                        for hi in range(2):
                            mm = nc.tensor.matmul(avs2[hi][:, 0:p_w],
                                                  v_stat(p_dsi, p_idx, a, hi),
                                                  p_ex[:, hi, 0:p_w],
                                                  start=p_first, stop=False,
                                                  skip_group_check=True)
                            if use_remote and p_dsi > 0:
                                rdma_waiters.append((mm, sem_v[p_dsi - 1]))
                    pend = (dsi, idx, w, ex, s_i == 0)
                p_dsi, p_idx, p_w, p_ex, p_first = pend
                for hi in range(2):
                    mm = nc.tensor.matmul(avs2[hi][:, 0:p_w],
                                          v_stat(p_dsi, p_idx, a, hi),
                                          p_ex[:, hi, 0:p_w],
                                          start=p_first, stop=True,
                                          skip_group_check=True)
                    if use_remote and p_dsi > 0:
                        rdma_waiters.append((mm, sem_v[p_dsi - 1]))

                for hi in range(2):
                    h = 2 * a + hi
                    avs = avs_pool.tile([65, RT], F32, tag="avs")
                    nc.scalar.copy(out=avs, in_=avs2[hi])
                    for i4 in range(NT):
                        pt = mm_ps.tile([128, 128], F32, tag="mm")
                        nc.tensor.transpose(pt[:, 0:65], avs[:, 128 * i4:128 * (i4 + 1)],
                                            ident_f[0:65, 0:65])
                        rec = stats.tile([128, 1], F32, tag="rec")
                        nc.vector.reciprocal(out=rec, in_=pt[:, 64:65])
                        nc.vector.tensor_scalar_mul(
                            out=attn_sb[:, i4, 64 * h:64 * (h + 1)],
                            in0=pt[:, 0:64], scalar1=rec)

            # ---- residual + LN2 -> h2, transpose -> h2T ----
            g2_r = None
            if affine:
                g2_r = const.tile([128, 2, C], F32, tag="lnr")
                nc.sync.dma_start(out=g2_r[:, 0], in_=bcast_row(g2_d[:]))
                nc.sync.dma_start(out=g2_r[:, 1], in_=bcast_row(be2_d[:]))
            h2 = big.tile([128, NT, C], BF16, tag="h_row")
            h2T = big.tile([128, NC_F, RT], BF16, tag="hT")
            for t in range(NT):
                nc.vector.tensor_add(out=x_sb[:, t], in0=x_sb[:, t], in1=attn_sb[:, t])
                layernorm(x_sb[:, t], h2[:, t], g2_r)
                for fc in range(NC_F):
                    pt = mm_ps.tile([128, 128], BF16, tag="mm")
                    nc.tensor.transpose(pt, h2[:, t, 128 * fc:128 * (fc + 1)], ident_b)
                    nc.vector.tensor_copy(out=h2T[:, fc, 128 * t:128 * (t + 1)], in_=pt)

            # ---- FFN1 + gelu -> g1T ----
            g1T = big.tile([128, NF_F, RT], BF16, tag="g1T")
            for fo in range(NF_F):
                if fo < 2:
                    w1_t = w1_pre[fo]
                else:
                    w1_t = w1s.tile([128, NC_F, 128], BF16, tag="w1t")
                    nc.sync.dma_start(out=w1_t,
                                      in_=w1_view[:, :, 128 * fo:128 * (fo + 1)])
                ps = mm_ps.tile([128, RT], F32, tag="mm")
                for ci in range(NC_F):
                    nc.tensor.matmul(ps, w1_t[:, ci, :], h2T[:, ci, :],
                                     start=(ci == 0), stop=(ci == NC_F - 1))
                nc.scalar.activation(out=g1T[:, fo, :], in_=ps, func=AF.Gelu,
                                     bias=b1_t[:, fo:fo + 1])

            # ---- FFN2 + residual -> y (W2 streamed in quarters) ----
            if affine:
                b2_r = const.tile([128, C], F32, tag="rowr")
                nc.sync.dma_start(out=b2_r, in_=bcast_row(b2_d[:]))
            w2_view = w2_d.rearrange("(a p) c -> p a c", p=128)
            y_view = y_d.rearrange("(t p) c -> p t c", p=128)
            for q4 in range(4):
                w2_sb = w2s.tile([128, NF_F, 256], BF16, tag="w2q")
                nc.sync.dma_start(out=w2_sb,
                                  in_=w2_view[:, :, 256 * q4:256 * (q4 + 1)])
                for t in range(NT):
                    ps = mm_ps.tile([128, 256], F32, tag="mm")
                    for fo in range(NF_F):
                        nc.tensor.matmul(ps, g1T[:, fo, 128 * t:128 * (t + 1)],
                                         w2_sb[:, fo, :],
                                         start=(fo == 0), stop=(fo == NF_F - 1))
                    dst = exs.tile([128, 256], F32, tag="ystage", bufs=4)
                    nc.vector.tensor_tensor(out=dst, in0=ps,
                                            in1=x_sb[:, t, 256 * q4:256 * (q4 + 1)],
                                            op=OP.add)
                    if affine:
                        nc.vector.tensor_add(out=dst, in0=dst,
                                             in1=b2_r[:, 256 * q4:256 * (q4 + 1)])
                    nc.sync.dma_start(out=y_view[:, t, 256 * q4:256 * (q4 + 1)],
                                      in_=dst)

    # attach remote-arrival waits AFTER Tile scheduling (the single-core
    # scheduling sim cannot model cross-core sem increments and would
    # deadlock on standalone waits); attaching to every reader makes the
    # gating robust to any instruction reordering.
    if use_remote:
        for inst, sem in rdma_waiters:
            inst.wait_op(sem, 4, "sem-ge", check=False)
    nc.compile()
    return nc


_NC_CACHE = {}


def _get_nc(affine=True):
    key = (affine, USE_REMOTE)
    if key not in _NC_CACHE:
        _NC_CACHE[key] = build_nc(affine=affine, use_remote=USE_REMOTE)
    return _NC_CACHE[key]


def _affine_trivial(inputs):
    one = lambda a: np.allclose(np.asarray(a, np.float32), 1.0)
    zero = lambda a: not np.any(np.asarray(a, np.float32))
    return (one(inputs["ln1_g"]) and zero(inputs["ln1_b"])
            and one(inputs["ln2_g"]) and zero(inputs["ln2_b"])
            and zero(inputs["bv"]) and zero(inputs["b2"]))


def _make_masks(p):
    """Masks for core with group-rank p: [NSLOT, 128, 128] bf16.

    Slot s = 4*dsi + oi processes source dsi (XOR distance if remote else
    rank), chunk index idx = SLOT_IDX_ORDER[oi]; the masked i-position block
    is k = slot_n(idx) - 1."""
    own = ownchunks(p)
    m = np.empty((NSLOT, 128, 128), np.float32)
    jj = np.arange(128)[:, None]
    ii = np.arange(128)[None, :]
    for dsi in range(4):
        src = (p ^ dsi) if USE_REMOTE else dsi
        for oi, idx in enumerate(SLOT_IDX_ORDER):
            cj = ownchunks(src)[idx]
            k = slot_n(idx) - 1
            ci = own[k]
            m[4 * dsi + oi] = (128 * cj + jj <= 128 * ci + ii)
    return m.astype(ml_dtypes.bfloat16)


def _prep_in_maps(inputs):
    x = np.asarray(inputs["x"], np.float32)
    cast_b = lambda a: np.asarray(np.asarray(a, np.float32)).astype(ml_dtypes.bfloat16)
    cast_f = lambda a: np.ascontiguousarray(np.asarray(a, np.float32))
    common = {
        "Wq": cast_b(inputs["Wq"]), "Wk": cast_b(inputs["Wk"]),
        "Wv": cast_b(inputs["Wv"]), "W1": cast_b(inputs["W1"]),
        "W2": cast_b(inputs["W2"]),
        "bq": cast_f(inputs["bq"]), "bk": cast_f(inputs["bk"]),
        "bv": cast_f(inputs["bv"]), "b1": cast_f(inputs["b1"]),
        "b2": cast_f(inputs["b2"]),
        "ln1_g": cast_f(inputs["ln1_g"]), "ln1_b": cast_f(inputs["ln1_b"]),
        "ln2_g": cast_f(inputs["ln2_g"]), "ln2_b": cast_f(inputs["ln2_b"]),
    }
    in_maps = []
    for core in range(N_CORES):
        b, p = core // 4, core % 4
        mp = dict(common)
        mp["x_loc"] = np.ascontiguousarray(
            np.concatenate([x[b, 128 * c:128 * (c + 1)] for c in ownchunks(p)])
        ).astype(ml_dtypes.bfloat16)
        mp["masks"] = _make_masks(p)
        in_maps.append(mp)
    return in_maps


def _assemble(results):
    out = np.empty((B, T, C), np.float32)
    for core in range(N_CORES):
        b, p = core // 4, core % 4
        y = results[core]["y"]
        for k, c in enumerate(ownchunks(p)):
            out[b, 128 * c:128 * (c + 1)] = y[128 * k:128 * (k + 1)]
    return out


_USED = [False]


def run_spmd(inputs, **kw):
    """Run on hardware; returns (full_output, BassKernelResults)."""
    in_maps = _prep_in_maps(inputs)
    affine = not _affine_trivial(inputs)
    if _USED[0]:
        # rebuild so a fresh NEFF load resets semaphore state
        _NC_CACHE.pop((affine, USE_REMOTE), None)
    nc = _get_nc(affine=affine)
    _USED[0] = True
    res = run_bass_kernel_spmd(nc, in_maps, core_ids=list(range(N_CORES)), **kw)
    return _assemble(res.results), res


def kernel(**inputs):
    out, _ = run_spmd(inputs)
    return out
